# revision 3
# baseline (speedup 1.0000x reference)
"""MoE gate (DeepSeek-style noaux_tc routing) Trainium2 kernel, v2.

Architecture (per core, 2048 tokens):
  - Host ships x pre-transposed and split into a bf16 pair (xh + xe ~ x to
    ~16 bits), and wT split likewise (wh + we). Rounding error of the
    3-term GEMM  logitsT = whT.T@xh + weT.T@xh + whT.T@xe  is ~2^-17
    relative; measured routing flip rate on the fixed harness inputs is
    2 tokens / 16384 (rel err 2.8e-3 vs the 2e-2 gate).
  - Expert-major GEMM: stationary = w chunk [128h, 128e], moving = xT
    [128h, 512t], accumulating logitsT [128e, 512t] in 4 PSUM banks
    (2 token-blocks x 2 expert-halves) per 1024-token half.
  - logitsT -> SBUF copy (DVE) -> PE transpose back to [128t, 256e] ->
    sigmoid + group-limited top-8 routing (ported from v1, DVE max8 /
    max_index + iota-match bias gather), sum-normalized * 2.5.
"""
import sys
sys.path.insert(0, "/opt/trn_rl_repo")
import numpy as np
import concourse.bass as bass
import concourse.tile as tile
from concourse import bacc, mybir
import ml_dtypes

F32 = mybir.dt.float32
BF16 = mybir.dt.bfloat16
U32 = mybir.dt.uint32
I32 = mybir.dt.int32
AF = mybir.ActivationFunctionType
ALU = mybir.AluOpType
AX = mybir.AxisListType

H = 7168
E = 256
NG_EXP = 8      # expert groups
GS = E // NG_EXP  # group size (32)
NCH = H // 128  # 56 h-chunks
CG = 4          # h-chunks per x DMA group
NGRP = NCH // CG
TCORE = 2048
NQ = 4          # 512-token quarters per core batch
BIG = 1.0e30


def _build(t_core=TCORE, n_devices=8, repeat=1, ldwopt=False, mode='full'):
    assert t_core == TCORE
    nc = bacc.Bacc("TRN2", target_bir_lowering=False, debug=False,
                   num_devices=n_devices)

    # packed pair, quarter-major: [q, chunk, partition, part(h/e), 512 tokens]
    xp_d = nc.dram_tensor("xp", [NQ, NCH, 128, 2, 512], BF16,
                          kind="ExternalInput")
    wh_d = nc.dram_tensor("wh", [NCH, 128, E], BF16, kind="ExternalInput")
    we_d = nc.dram_tensor("we", [NCH, 128, E], BF16, kind="ExternalInput")
    bias_d = nc.dram_tensor("bias_b", [128, E], F32, kind="ExternalInput")
    iota_d = nc.dram_tensor("iota_b", [128, E], F32, kind="ExternalInput")
    ident_d = nc.dram_tensor("ident", [128, 128], F32, kind="ExternalInput")
    idx_d = nc.dram_tensor("idx_out", [t_core, 8], I32, kind="ExternalOutput")
    w_d = nc.dram_tensor("w_out", [t_core, 8], F32, kind="ExternalOutput")

    xp_v = xp_d[:].rearrange("q c p s t -> p q c s t")
    wh_v = wh_d[:].rearrange("c p e -> p c e")
    we_v = we_d[:].rearrange("c p e -> p c e")

    with tile.TileContext(nc) as tc:
        with (
            tc.tile_pool(name="const", bufs=1) as constp,
            tc.tile_pool(name="wsb", bufs=2) as wp,
            tc.tile_pool(name="xg", bufs=3) as xg,
            tc.tile_pool(name="lsb", bufs=2) as lp,
            tc.tile_pool(name="route", bufs=2) as rp,
            tc.tile_pool(name="small", bufs=2) as sp,
            tc.tile_pool(name="ltps", bufs=2, space="PSUM") as ltps,
            tc.tile_pool(name="tps", bufs=2, space="PSUM") as tps,
        ):
            ident = constp.tile([128, 128], F32)
            bias_sb = constp.tile([128, E], F32)
            iota_sb = constp.tile([128, E], F32)
            bias_bf = constp.tile([128, E], BF16)
            iota_bf = constp.tile([128, E], BF16)

            def pe_touch(ap):
                # PE no-op "writing" `ap`: absorbs the cross-engine semaphore
                # wait (and orders before all readers) so the following
                # Ldweights carries none (required by walrus
                # --enable-ldw-opt).
                nc.tensor.add_instruction(
                    mybir.InstNoOp(
                        name=nc.get_next_instruction_name(),
                        ins=[], outs=[nc.tensor.lower_ap(ap)],
                        text_hint="ldw_absorb"))

            def emit_routing(i, logits):
                """Routing for 128-token sub-tile i; logits [128, 256] PSUM."""
                scores = rp.tile([128, E], F32, tag="scores",
                                 name=f"scores_{i}")
                nc.scalar.activation(scores[:], logits[:], AF.Sigmoid)
                if mode == "noroute":
                    nc.gpsimd.dma_start(idx_d[128 * i:128 * (i + 1), :],
                                        scores[:, 0:8].bitcast(I32))
                    nc.gpsimd.dma_start(w_d[128 * i:128 * (i + 1), :],
                                        scores[:, 8:16])
                    return
                sfc = rp.tile([128, E], F32, tag="sfc", name=f"sfc_{i}")
                nc.vector.tensor_tensor(sfc[:], scores[:], bias_sb[:],
                                        op=ALU.add)

                g8 = sp.tile([128, 64], F32, tag="g8", name=f"g8_{i}")
                for g in range(NG_EXP):
                    nc.vector.max(g8[:, 8 * g:8 * g + 8],
                                  sfc[:, GS * g:GS * (g + 1)])
                gsc = sp.tile([128, NG_EXP], F32, tag="gsc", name=f"gsc_{i}")
                nc.vector.tensor_reduce(
                    gsc[:],
                    g8[:].rearrange("p (g i) -> p g i", i=8)[:, :, 0:2],
                    axis=AX.X, op=ALU.add)

                gt8 = sp.tile([128, 8], F32, tag="gt8", name=f"gt8_{i}")
                nc.vector.max(gt8[:], gsc[:])
                pen = sp.tile([128, NG_EXP], F32, tag="pen", name=f"pen_{i}")
                nc.vector.tensor_scalar(pen[:], gsc[:], gt8[:, 3:4], -BIG,
                                        op0=ALU.is_lt, op1=ALU.mult)

                masked = rp.tile([128, E], F32, tag="masked",
                                 name=f"masked_{i}")
                for g in range(NG_EXP):
                    nc.vector.tensor_scalar_add(
                        masked[:, GS * g:GS * (g + 1)],
                        sfc[:, GS * g:GS * (g + 1)], pen[:, g:g + 1])

                m8 = sp.tile([128, 8], F32, tag="m8", name=f"m8_{i}")
                nc.vector.max(m8[:], masked[:])
                i8 = sp.tile([128, 8], U32, tag="i8", name=f"i8_{i}")
                nc.vector.max_index(i8[:], m8[:], masked[:])

                i8f = sp.tile([128, 8], BF16, tag="i8f", name=f"i8f_{i}")
                nc.vector.tensor_copy(i8f[:], i8[:])
                junk = rp.tile([128, E], BF16, tag="junk", name=f"junk_{i}")
                biasg = sp.tile([128, 8], F32, tag="biasg",
                                name=f"biasg_{i}")
                for k in range(8):
                    nc.vector.scalar_tensor_tensor(
                        junk[:], iota_bf[:], i8f[:, k:k + 1], bias_bf[:],
                        op0=ALU.is_equal, op1=ALU.mult,
                        accum_out=biasg[:, k:k + 1])

                wraw = sp.tile([128, 8], F32, tag="wraw", name=f"wraw_{i}")
                nc.vector.tensor_tensor(wraw[:], m8[:], biasg[:],
                                        op=ALU.subtract)
                ssum = sp.tile([128, 1], F32, tag="ssum", name=f"ssum_{i}")
                nc.vector.tensor_reduce(ssum[:], wraw[:], axis=AX.X,
                                        op=ALU.add)
                inv = sp.tile([128, 1], F32, tag="inv", name=f"inv_{i}")
                nc.vector.reciprocal(inv[:], ssum[:])
                wout = sp.tile([128, 8], F32, tag="wout", name=f"wout_{i}")
                nc.vector.tensor_scalar(wout[:], wraw[:], inv[:], 2.5,
                                        op0=ALU.mult, op1=ALU.mult)

                nc.gpsimd.dma_start(idx_d[128 * i:128 * (i + 1), :],
                                    i8[:].bitcast(I32))
                nc.gpsimd.dma_start(w_d[128 * i:128 * (i + 1), :], wout[:])

            def emit():
                nc.sync.dma_start(ident[:], ident_d[:])
                nc.gpsimd.dma_start(bias_sb[:], bias_d[:])
                nc.gpsimd.dma_start(iota_sb[:], iota_d[:])
                nc.vector.tensor_copy(bias_bf[:], bias_sb[:])
                nc.vector.tensor_copy(iota_bf[:], iota_sb[:])
                wh_sb = wp.tile([128, NCH, E], BF16, tag="wh_sb")
                we_sb = wp.tile([128, NCH, E], BF16, tag="we_sb")
                for g in range(NGRP):
                    sl = slice(CG * g, CG * (g + 1))
                    nc.gpsimd.dma_start(wh_sb[:, sl, :], wh_v[:, sl, :])
                    nc.gpsimd.dma_start(we_sb[:, sl, :], we_v[:, sl, :])
                    if ldwopt:
                        pe_touch(wh_sb[:, sl, :])
                        pe_touch(we_sb[:, sl, :])

                def emit_quarter_gemm(q, after_group1=None):
                    lt = {}
                    for eh in range(2):
                        lt[eh] = ltps.tile([128, 512], F32, tag=f"lt_{eh}",
                                           name=f"lt_{q}_{eh}")
                    for g in range(NGRP):
                        if g == 2 and after_group1 is not None:
                            after_group1()
                        csl = slice(CG * g, CG * (g + 1))
                        xpg = xg.tile([128, CG, 2, 512], BF16, tag="xpg",
                                      name=f"xpg_{q}_{g}")
                        ring = nc.sync if g % 2 == 0 else nc.scalar
                        ring.dma_start(xpg[:], xp_v[:, q, csl, :, :])
                        for cc in range(CG):
                            c = CG * g + cc
                            first = (c == 0)
                            last = (c == NCH - 1)
                            for eh in range(2):
                                esl = slice(128 * eh, 128 * (eh + 1))
                                for s in (0, 1):
                                    nc.tensor.matmul(
                                        lt[eh][:], wh_sb[:, c, esl],
                                        xpg[:, cc, s, :],
                                        start=(first and s == 0),
                                        stop=(last and s == 1))
                                nc.tensor.matmul(
                                    lt[eh][:], we_sb[:, c, esl],
                                    xpg[:, cc, 0, :],
                                    start=False, stop=False)
                    return lt

                def emit_quarter_drain(q, lt):
                    lsb = {}
                    for eh in range(2):
                        lsb[eh] = lp.tile([128, 512], F32, tag=f"lsb_{eh}",
                                          name=f"lsb_{q}_{eh}")
                        nc.scalar.copy(lsb[eh][:], lt[eh][:])
                    if mode == "gemmonly":
                        # ship raw logits halves out instead of routing
                        nc.gpsimd.dma_start(
                            idx_d[512 * q:512 * q + 128, :],
                            lsb[0][:, 0:8].bitcast(I32))
                        nc.gpsimd.dma_start(
                            w_d[512 * q:512 * q + 128, :], lsb[1][:, 0:8])
                        return
                    for st in range(4):
                        i = q * 4 + st
                        lg = tps.tile([128, E], F32, tag=f"lg_{st % 2}",
                                      name=f"lg_{q}_{st}")
                        for eh in range(2):
                            nc.tensor.transpose(
                                lg[:, 128 * eh:128 * (eh + 1)],
                                lsb[eh][:, 128 * st:128 * (st + 1)],
                                ident[:])
                        emit_routing(i, lg)

                pending = None
                for q in range(NQ):
                    if pending is not None:
                        pq, plt = q - 1, pending
                        cb = lambda pq=pq, plt=plt: emit_quarter_drain(pq, plt)
                    else:
                        cb = None
                    lt = emit_quarter_gemm(q, after_group1=cb)
                    pending = lt
                emit_quarter_drain(NQ - 1, pending)

            if repeat == 1:
                emit()
            else:
                with tc.For_i(0, repeat, 1):
                    emit()
    nc.compile()
    return nc


_NC_CACHE = {}
_T_FULL = 16384
_N_CORES = 8


def _host_prep(x_full, w, bias):
    """x_full [T, H] f32, w [E, H] f32, bias [E] f32 -> per-core maps."""
    xh = x_full.astype(ml_dtypes.bfloat16)
    xe = (x_full - xh.astype(np.float32)).astype(ml_dtypes.bfloat16)
    wT = np.ascontiguousarray(w.T)  # [H, E]
    wh = wT.astype(ml_dtypes.bfloat16)
    we = (wT - wh.astype(np.float32)).astype(ml_dtypes.bfloat16)
    base = {
        "wh": np.ascontiguousarray(wh.reshape(NCH, 128, E)),
        "we": np.ascontiguousarray(we.reshape(NCH, 128, E)),
        "bias_b": np.ascontiguousarray(
            np.broadcast_to(bias[None, :], (128, E))),
        "iota_b": np.ascontiguousarray(
            np.broadcast_to(np.arange(E, dtype=np.float32)[None, :],
                            (128, E))),
        "ident": np.eye(128, dtype=np.float32),
    }
    maps = []
    for c in range(_N_CORES):
        m = dict(base)
        sl = slice(c * TCORE, (c + 1) * TCORE)
        # [T, H] -> xT [H, T] -> [NCH, 128, NQ, 512]
        xhT = xh[sl].T.reshape(NCH, 128, NQ, 512)
        xeT = xe[sl].T.reshape(NCH, 128, NQ, 512)
        # stack parts -> [NCH, 128, q, part, 512] -> [q, NCH, 128, part, 512]
        xp = np.stack([xhT, xeT], axis=3).transpose(2, 0, 1, 3, 4)
        m["xp"] = np.ascontiguousarray(xp)
        maps.append(m)
    return maps


def kernel(hidden_states, weight, e_score_correction_bias):
    from concourse.bass_utils import run_bass_kernel_spmd

    x = np.ascontiguousarray(
        np.asarray(hidden_states, dtype=np.float32).reshape(_T_FULL, H))
    w = np.asarray(weight, dtype=np.float32)
    bias = np.asarray(e_score_correction_bias, dtype=np.float32)

    if "nc" not in _NC_CACHE:
        _NC_CACHE["nc"] = _build(TCORE, n_devices=_N_CORES)
    nc = _NC_CACHE["nc"]

    maps = _host_prep(x, w, bias)
    br = run_bass_kernel_spmd(nc, maps, list(range(_N_CORES)))
    idx = np.concatenate(
        [br.results[c]["idx_out"] for c in range(_N_CORES)],
        axis=0).astype(np.int32)
    wout = np.concatenate(
        [br.results[c]["w_out"] for c in range(_N_CORES)],
        axis=0).astype(np.float32)
    return idx, wout
